# revision 1
# baseline (speedup 1.0000x reference)
"""Multi-head causal attention (B=2, S=2048, D=1024, H=16) on 8 trn2 NeuronCores.

Sharding: data-parallel over batch (2) x tensor-parallel over heads (4 groups of
4 heads).  Core c = 4*b + g handles batch b, heads [4g, 4g+4).  Each core
computes a partial output  ctx_g @ Wo_g.T  [2048, 1024]; the host sums the 4
partials per batch.

Within-core dataflow:
  qT,kT = W @ X.T        float32r matmuls (fp32 operands, fp22 datapath, full
                         PE rate at moving-dim >= 256); [dk, s] pair-packed
  v     = X @ Wv.T       natural [s, dv], stored bf16
  S     = qT.T @ kT      per 128-row query tile, causally-trimmed segments
                         grouped into <=1024-wide PSUM tiles; two heads
                         row-packed via tile_position
  P     = exp(8S - 8m)   m = per-group local max (no serial chain); exp on
                         ScalarE writes bf16 P, accum_out gives group sums;
                         global rescale exp(8(m_g - m*))/Z folded into one
                         gpsimd pass over P
  P_T   = PE transpose   bf16 128x128 blocks into 512-wide q supertiles
  ctxT  = v.T @ P_T      bf16, two heads col-packed, accumulated over k tiles
  out  += ctxT.T @ WoT   float32r, per s-tile, stored straight from PSUM
"""

import numpy as np

B, S, D, H = 2, 2048, 1024, 16
DK = D // H          # 64
JC = 256             # per-core projection width (4 heads * 64)
NQT = S // 128       # 16 query tiles
NU4 = S // 512       # 4 query supertiles
_SCALE = float(DK) ** 0.5  # 8.0  (reference multiplies scores by sqrt(dk))
_MASKVAL = -1.0e30

_cached = {}
_BUILD_STAGES = "full"  # debug: dma | proj | noattn | full


def _build_nc(reps=1):
    stages = _BUILD_STAGES
    from contextlib import ExitStack

    import concourse.mybir as mybir
    import concourse.tile as tile
    from concourse import bacc

    F32 = mybir.dt.float32
    F32R = mybir.dt.float32r
    BF16 = mybir.dt.bfloat16
    EXP = mybir.ActivationFunctionType.Exp
    AX = mybir.AxisListType.X
    MIN = mybir.AluOpType.min

    nc = bacc.Bacc("TRN2", target_bir_lowering=False)

    xtq_d = nc.dram_tensor("xtq", [D, S], F32R, kind="ExternalInput")
    xtk_d = nc.dram_tensor("xtk", [D, S], F32R, kind="ExternalInput")
    xtv_d = nc.dram_tensor("xtv", [D, S], F32R, kind="ExternalInput")
    wqt_d = nc.dram_tensor("wqt", [D, JC], F32R, kind="ExternalInput")
    wkt_d = nc.dram_tensor("wkt", [D, JC], F32R, kind="ExternalInput")
    wvt_d = nc.dram_tensor("wvt", [D, JC], F32R, kind="ExternalInput")
    wot_d = nc.dram_tensor("wot", [JC, D], F32R, kind="ExternalInput")
    cmask_d = nc.dram_tensor("cmask", [128, 128], F32, kind="ExternalInput")
    ident_d = nc.dram_tensor("ident", [128, 128], BF16, kind="ExternalInput")
    out_d = nc.dram_tensor("out", [S, D], F32, kind="ExternalOutput")

    with tile.TileContext(nc) as tc, ExitStack() as top:
        res = top.enter_context(tc.tile_pool(name="res", bufs=1))
        stats = top.enter_context(tc.tile_pool(name="stats", bufs=1))

        # ---- resident tiles -------------------------------------------------
        # weight layouts: w?_sb[p, dt, j] = W?T[128*dt + p, j]
        wq_sb = res.tile([128, 8, JC], F32R, tag="wq")
        wk_sb = res.tile([128, 8, JC], F32R, tag="wk")
        wv_sb = res.tile([128, 8, JC], F32R, tag="wv")
        nc.sync.dma_start(wq_sb, wqt_d[:, :].rearrange("(t p) j -> p t j", p=128))
        nc.sync.dma_start(wk_sb, wkt_d[:, :].rearrange("(t p) j -> p t j", p=128))
        nc.sync.dma_start(wv_sb, wvt_d[:, :].rearrange("(t p) j -> p t j", p=128))
        wo_sb = []
        for p2 in range(2):
            t = res.tile([128, D], F32R, tag=f"wo{p2}", name=f"wo{p2}")
            nc.sync.dma_start(t, wot_d[128 * p2 : 128 * (p2 + 1), :])
            wo_sb.append(t)
        cmask = res.tile([128, 128], F32, tag="cmask")
        ident = res.tile([128, 128], BF16, tag="ident")
        nc.sync.dma_start(cmask, cmask_d[:, :])
        nc.sync.dma_start(ident, ident_d[:, :])

        # projected tensors (resident through attention), segmented 512-wide so
        # Tile's per-tile dependency tracking lets attention start on early
        # segments while later projection chunks are still in flight
        qseg = [[res.tile([128, 512], F32R, tag=f"qts{i}{c}", name=f"qts{i}{c}")
                 for c in range(4)] for i in range(2)]
        kseg = [[res.tile([128, 512], F32R, tag=f"kts{i}{c}", name=f"kts{i}{c}")
                 for c in range(4)] for i in range(2)]
        vu = [res.tile([128, JC], BF16, tag=f"vu{i}", name=f"vu{i}") for i in range(NQT)]
        ctxseg = [[res.tile([128, 512], F32R, tag=f"ctx{i}{c}", name=f"ctx{i}{c}")
                   for c in range(4)] for i in range(2)]

        def _one_pass(_rep):
            # ---- stage B: projections --------------------------------------
            with ExitStack() as stage_b:
                xpool = stage_b.enter_context(tc.tile_pool(name=f"xt{_rep}", bufs=1))
                pjp = stage_b.enter_context(
                    tc.tile_pool(name=f"pj{_rep}", bufs=1, space="PSUM")
                )
                CH = 512
                work = [(xtq_d, "q", 0), (xtk_d, "k", 0), (xtq_d, "q", 1),
                        (xtk_d, "k", 1), (xtq_d, "q", 2), (xtk_d, "k", 2),
                        (xtq_d, "q", 3), (xtk_d, "k", 3)]
                work += [(xtv_d, "v", c) for c in range(4)]
                for n_, (xd, kind, ch) in enumerate(work):
                    sl = slice(ch * CH, (ch + 1) * CH)
                    xc = xpool.tile([128, 8, CH], F32R, tag="xc", bufs=3, name="xc")
                    dmae = nc.sync if n_ % 2 == 0 else nc.scalar
                    dmae.dma_start(
                        xc, xd[:, sl].rearrange("(t p) s -> p t s", p=128)
                    )
                    if stages == "dma":
                        dd = stats.tile([128, 1], F32, tag="dd", bufs=8, name="dd")
                        nc.vector.reduce_max(
                            out=dd, in_=xc[:, 0, :].bitcast(F32), axis=AX
                        )
                        nc.sync.dma_start(out_d[0:128, n_ : n_ + 1], dd)
                        continue
                    if kind in ("q", "k"):
                        wsb = wq_sb if kind == "q" else wk_sb
                        dst = qseg if kind == "q" else kseg
                        for jt in range(2):
                            ps = pjp.tile([128, CH], F32, tag="pj", bufs=3,
                                          name="psqk")
                            for dt in range(8):
                                nc.tensor.matmul(
                                    ps,
                                    wsb[:, dt, 128 * jt : 128 * (jt + 1)],
                                    xc[:, dt, :],
                                    start=(dt == 0),
                                    stop=(dt == 7),
                                )
                            if (ch + jt) % 2 == 0:
                                nc.vector.tensor_copy(dst[jt][ch], ps)
                            else:
                                nc.scalar.copy(dst[jt][ch], ps)
                    else:
                        for st in range(4):
                            ps = pjp.tile([128, JC], F32, tag="pjv", bufs=2,
                                          name="psv")
                            for dt in range(8):
                                nc.tensor.matmul(
                                    ps,
                                    xc[:, dt, st * 128 : (st + 1) * 128],
                                    wv_sb[:, dt, :],
                                    start=(dt == 0),
                                    stop=(dt == 7),
                                )
                            if st % 2 == 0:
                                nc.vector.tensor_copy(vu[4 * ch + st], ps)
                            else:
                                nc.scalar.copy(vu[4 * ch + st], ps)

            if stages == "dma":
                return
            if stages == "proj":
                nc.sync.dma_start(out_d[0:128, :], qseg[0][0][:, 0:1024].bitcast(F32))
                nc.sync.dma_start(out_d[128:256, :], kseg[1][0][:, 0:1024].bitcast(F32))
                return

            # ---- stage C/D: attention + output projection ------------------
            with ExitStack() as stage_c:
                ppool = stage_c.enter_context(tc.tile_pool(name=f"pp{_rep}", bufs=1))
                ptp = stage_c.enter_context(tc.tile_pool(name=f"ptp{_rep}", bufs=1))
                obp = stage_c.enter_context(tc.tile_pool(name=f"obp{_rep}", bufs=1))
                pss_p = stage_c.enter_context(
                    tc.tile_pool(name=f"pss{_rep}", bufs=1, space="PSUM"))
                pst_p = stage_c.enter_context(
                    tc.tile_pool(name=f"pst{_rep}", bufs=1, space="PSUM"))
                psc_p = stage_c.enter_context(
                    tc.tile_pool(name=f"psc{_rep}", bufs=1, space="PSUM"))
                pso_p = stage_c.enter_context(
                    tc.tile_pool(name=f"pso{_rep}", bufs=1, space="PSUM"))

                ncopy = 0  # round-robin DVE/ACT for PSUM->SBUF copies

                for u in range(NU4):
                    for p in range(2):
                        ptiles = {}
                        for sq in range(4):
                            qi = 4 * u + sq
                            qsl = slice(qi * 128, (qi + 1) * 128)
                            nfull = qi // 4
                            segs = [(512 * j, 512) for j in range(nfull)]
                            segs.append((512 * nfull, 128 * (qi % 4) + 128))
                            nch = len(segs)  # <= 4
                            for h in range(2):
                                pt = ppool.tile([128, S], BF16, tag="P", bufs=10,
                                                name=f"P{p}{sq}{h}")
                                ptiles[(h, sq)] = pt
                                hsl = slice(64 * h, 64 * (h + 1))
                                stt = stats.tile([128, 12], F32, tag="st", bufs=16,
                                                 name="stt")
                                for g, (off, w) in enumerate(segs):
                                    ps = pss_p.tile([128, 512], F32, tag="pss",
                                                    bufs=4, name="pss")
                                    nc.tensor.matmul(
                                        ps[:, 0:w],
                                        qseg[p][qi // 4][hsl, 128 * (qi % 4) : 128 * (qi % 4) + 128],
                                        kseg[p][off // 512][hsl, 0:w],
                                        start=True,
                                        stop=True,
                                        tile_position=(64 * h, 0),
                                    )
                                    if g == nch - 1:  # diagonal chunk
                                        nc.vector.tensor_add(
                                            ps[:, w - 128 : w],
                                            ps[:, w - 128 : w],
                                            cmask,
                                        )
                                    cmx = stats.tile([128, 1], F32, tag="cm", bufs=16,
                                                     name="cmx")
                                    nc.vector.reduce_max(out=cmx, in_=ps[:, 0:w],
                                                         axis=AX)
                                    nc.vector.tensor_scalar_mul(
                                        stt[:, g : g + 1], cmx, -_SCALE
                                    )
                                    if stages == "scores":
                                        continue
                                    nc.scalar.activation(
                                        out=pt[:, off : off + w],
                                        in_=ps[:, 0:w],
                                        func=EXP,
                                        bias=stt[:, g : g + 1],
                                        scale=_SCALE,
                                        accum_out=stt[:, 4 + g : 5 + g],
                                    )
                                if stages in ("scores", "exphalf"):
                                    continue
                                # per-chunk scale = exp(8(m_g - m*)) / Z*
                                if nch == 1:
                                    nc.vector.reciprocal(stt[:, 8:9], stt[:, 4:5])
                                    zft = stt[:, 8:9]
                                else:
                                    nc.vector.tensor_reduce(
                                        out=stt[:, 8:9], in_=stt[:, 0:nch],
                                        op=MIN, axis=AX,
                                    )
                                    zft = stats.tile([128, 4], F32, tag="zf", bufs=16,
                                                     name="zft")
                                    nc.scalar.activation(
                                        out=zft[:, 0:nch], in_=stt[:, 0:nch],
                                        func=EXP, bias=stt[:, 8:9], scale=-1.0,
                                    )
                                    nc.vector.tensor_mul(
                                        stt[:, 0:nch], zft[:, 0:nch], stt[:, 4 : 4 + nch]
                                    )
                                    nc.vector.reduce_sum(out=stt[:, 9:10],
                                                         in_=stt[:, 0:nch], axis=AX)
                                    nc.vector.reciprocal(stt[:, 10:11], stt[:, 9:10])
                                    nc.vector.tensor_scalar_mul(
                                        zft[:, 0:nch], zft[:, 0:nch], stt[:, 10:11]
                                    )
                                # normalalize P in place (DVE bf16 4x mode)
                                for g, (off, w) in enumerate(segs):
                                    zcol = zft if nch == 1 else zft[:, g : g + 1]
                                    nc.vector.tensor_scalar_mul(
                                        pt[:, off : off + w], pt[:, off : off + w],
                                        zcol,
                                    )
                        if stages in ("noattn", "scores", "exphalf"):
                            continue
                        # transposes + PV for this (pair, supertile)
                        psc = psc_p.tile([128, 512], F32, tag="psc", bufs=1,
                                         name=f"psc{p}{u}")
                        for t in range(4 * u + 4):
                            vstart = max(0, t - 4 * u)
                            csl = slice(128 * vstart, 512)
                            for h in range(2):
                                pstile = pst_p.tile([128, 512], BF16, tag="pst",
                                                    bufs=2, name="pst")
                                for sq in range(vstart, 4):
                                    nc.tensor.transpose(
                                        pstile[:, 128 * sq : 128 * (sq + 1)],
                                        ptiles[(h, sq)][:, 128 * t : 128 * (t + 1)],
                                        ident,
                                    )
                                ptsb = ptp.tile([128, 512], BF16, tag="pt", bufs=3,
                                                name="ptsb")
                                if ncopy % 2 == 0:
                                    nc.vector.tensor_copy(ptsb[:, csl], pstile[:, csl])
                                else:
                                    nc.scalar.copy(ptsb[:, csl], pstile[:, csl])
                                ncopy += 1
                                nc.tensor.matmul(
                                    psc[64 * h : 64 * (h + 1), csl],
                                    vu[t][:, 64 * (2 * p + h) : 64 * (2 * p + h + 1)],
                                    ptsb[:, csl],
                                    start=(t == 0),
                                    stop=(t == 4 * u + 3),
                                    tile_position=(0, 64 * h),
                                    skip_group_check=True,
                                )
                        if u % 2 == 0:
                            nc.vector.tensor_copy(ctxseg[p][u], psc)
                        else:
                            nc.scalar.copy(ctxseg[p][u], psc)
                    if stages in ("noattn", "scores", "exphalf"):
                        continue
                    # output projection for the four finished s-tiles
                    for st_ in range(4 * u, 4 * u + 4):
                        ssl = slice(128 * st_, 128 * (st_ + 1))
                        csl_ = slice(128 * (st_ % 4), 128 * (st_ % 4) + 128)
                        for oc in range(2):
                            osl = slice(512 * oc, 512 * (oc + 1))
                            pso = pso_p.tile([128, 512], F32, tag="pso", bufs=1,
                                             name="pso")
                            nc.tensor.matmul(pso, ctxseg[0][u][:, csl_],
                                             wo_sb[0][:, osl],
                                             start=True, stop=False)
                            nc.tensor.matmul(pso, ctxseg[1][u][:, csl_],
                                             wo_sb[1][:, osl],
                                             start=False, stop=True)
                            osb = obp.tile([128, 512], F32, tag="ob", bufs=3,
                                           name="osb")
                            if oc == 0:
                                nc.vector.tensor_copy(osb, pso)
                            else:
                                nc.scalar.copy(osb, pso)
                            nc.sync.dma_start(out_d[ssl, osl], osb)

        for _rep in range(reps):
            if _rep:
                tc.strict_bb_all_engine_barrier()
            _one_pass(_rep)

    nc.compile()
    return nc


def _get_nc(reps=1):
    key = ("nc", reps, _BUILD_STAGES)
    if key not in _cached:
        _cached[key] = _build_nc(reps)
    return _cached[key]


def _fp22(a):
    """Truncate fp32 to fp22 (e8m13) as the PE's float32r datapath does."""
    a = np.ascontiguousarray(a, dtype=np.float32)
    a.view(np.uint32)[...] &= np.uint32(0xFFFFFC00)
    return a


def _host_inputs(query, key, value, Wq, Wk, Wv, Wo):
    """Build the 8 per-core input dicts (host-side transposes/slices)."""
    f32 = np.float32
    xt = {}
    for b in range(B):
        xt[("q", b)] = _fp22(query[b].T)
        xt[("k", b)] = _fp22(key[b].T)
        xt[("v", b)] = _fp22(value[b].T)
    import ml_dtypes

    cmask = np.where(
        np.arange(128)[None, :] <= np.arange(128)[:, None], 0.0, _MASKVAL
    ).astype(f32)
    ident = np.eye(128).astype(ml_dtypes.bfloat16)
    in_maps = []
    for c in range(8):
        b, g = c // 4, c % 4
        jsl = slice(JC * g, JC * (g + 1))
        in_maps.append(
            {
                "xtq": xt[("q", b)],
                "xtk": xt[("k", b)],
                "xtv": xt[("v", b)],
                "wqt": _fp22(Wq[jsl, :].T),
                "wkt": _fp22(Wk[jsl, :].T),
                "wvt": _fp22(Wv[jsl, :].T),
                "wot": _fp22(Wo[:, jsl].T),
                "cmask": cmask,
                "ident": ident,
            }
        )
    return in_maps


def _numpy_fallback(query, key, value, mask, Wq, Wk, Wv, Wo):
    """Exact (chunked) numpy path for non-causal masks."""
    out = np.empty((B, S, D), dtype=np.float32)
    q = (query @ Wq.T).reshape(B, S, H, DK).transpose(0, 2, 1, 3)
    k = (key @ Wk.T).reshape(B, S, H, DK).transpose(0, 2, 1, 3)
    v = (value @ Wv.T).reshape(B, S, H, DK).transpose(0, 2, 1, 3)
    for b in range(B):
        ctx = np.empty((H, S, DK), dtype=np.float32)
        mb = mask[b] == 0
        for h in range(H):
            s = (q[b, h] @ k[b, h].T) * _SCALE
            s[mb] = np.finfo(np.float32).min
            s -= s.max(axis=1, keepdims=True)
            np.exp(s, out=s)
            s /= s.sum(axis=1, keepdims=True)
            ctx[h] = s @ v[b, h]
        out[b] = ctx.transpose(1, 0, 2).reshape(S, D) @ Wo.T
    return out


def kernel(query, key, value, mask, Wq, Wk, Wv, Wo):
    query = np.asarray(query, dtype=np.float32)
    key = np.asarray(key, dtype=np.float32)
    value = np.asarray(value, dtype=np.float32)
    mask = np.asarray(mask)
    Wq, Wk, Wv, Wo = (np.asarray(w, dtype=np.float32) for w in (Wq, Wk, Wv, Wo))

    tril = np.tril(np.ones((S, S), dtype=mask.dtype))
    if not all(np.array_equal(mask[b], tril) for b in range(B)):
        return _numpy_fallback(query, key, value, mask, Wq, Wk, Wv, Wo)

    from concourse.bass_utils import run_bass_kernel_spmd

    nc = _get_nc()
    in_maps = _host_inputs(query, key, value, Wq, Wk, Wv, Wo)
    res = run_bass_kernel_spmd(nc, in_maps, core_ids=list(range(8)))
    outs = [r["out"] for r in res.results]
    full = np.empty((B, S, D), dtype=np.float32)
    for b in range(B):
        full[b] = outs[4 * b] + outs[4 * b + 1] + outs[4 * b + 2] + outs[4 * b + 3]
    return full



# revision 11
# speedup vs baseline: 1.5610x; 1.5610x over previous
"""Multi-head causal attention (B=2, S=2048, D=1024, H=16) on 8 trn2 NeuronCores.

Sharding: data-parallel over batch (2) x tensor-parallel over heads (4 groups of
4 heads).  Core c = 4*b + g handles batch b, heads [4g, 4g+4).  Each core
computes a partial output  ctx_g @ Wo_g.T  [2048, 1024]; the host sums the 4
partials per batch.

Within-core dataflow (v2):
  qT,kT = W @ X.T        float32r matmuls; Wq pre-scaled by sqrt(dk)=8 on host
                         so scores come out of the PE already scaled
  v     = X @ Wv.T       natural [s, dv], stored bf16
  S     = qT.T @ kT      per 128-row query tile, contiguous 2-bank PSUM regions
                         (<=1024 wide); causal mask + padding written by a PE
                         matmul (ident x mask-tile) accumulated under the
                         diagonal score matmul; all score matmuls >=256 moving
  m     = rowmax         ONE negated reduce_max per region (DVE) -> exp bias
  P     = exp(S - m)     ONE exp per region on ACT, accum_out -> Z
  P    *= 1/Z            one DVE 4x bf16 pass per tile
  P_T   = PE transpose   bf16 128x128 blocks into 512-wide q supertiles
  ctxT  = v.T @ P_T      bf16, two heads col-packed, accumulated over k tiles
  out  += ctxT.T @ WoT   float32r, per s-tile, stored straight from PSUM
"""

import numpy as np

B, S, D, H = 2, 2048, 1024, 16
DK = D // H          # 64
JC = 256             # per-core projection width (4 heads * 64)
NQT = S // 128       # 16 query tiles
NU4 = S // 512       # 4 query supertiles
_SCALE = float(DK) ** 0.5  # 8.0  (folded into Wq on the host)
_MASKVAL = -1.0e30

_cached = {}

# knobs
_PTSB_PAT = (0, 1, 0, 0, 1)   # ptsb copy engine pattern: 0=DVE, 1=ACT
_PSS_BUFS = 2
_PT_BUFS = 18


def _build_nc(reps=1):
    from contextlib import ExitStack

    import concourse.mybir as mybir
    import concourse.tile as tile
    from concourse import bacc

    F32 = mybir.dt.float32
    F32R = mybir.dt.float32r
    BF16 = mybir.dt.bfloat16
    EXP = mybir.ActivationFunctionType.Exp
    AX = mybir.AxisListType.X
    MIN = mybir.AluOpType.min

    nc = bacc.Bacc("TRN2", target_bir_lowering=False)

    xtq_d = nc.dram_tensor("xtq", [D, S], F32R, kind="ExternalInput")
    xtk_d = nc.dram_tensor("xtk", [D, S], F32R, kind="ExternalInput")
    xtv_d = nc.dram_tensor("xtv", [D, S], F32R, kind="ExternalInput")
    wqt_d = nc.dram_tensor("wqt", [D, JC], F32R, kind="ExternalInput")
    wkt_d = nc.dram_tensor("wkt", [D, JC], F32R, kind="ExternalInput")
    wvt_d = nc.dram_tensor("wvt", [D, JC], F32R, kind="ExternalInput")
    wot_d = nc.dram_tensor("wot", [JC, D], F32R, kind="ExternalInput")
    maska_d = nc.dram_tensor("maska", [128, 256], BF16, kind="ExternalInput")
    maskb_d = nc.dram_tensor("maskb", [128, 256], BF16, kind="ExternalInput")
    ident_d = nc.dram_tensor("ident", [128, 128], BF16, kind="ExternalInput")
    out_d = nc.dram_tensor("out", [S, D], F32, kind="ExternalOutput")

    with tile.TileContext(nc) as tc, ExitStack() as top:
        res = top.enter_context(tc.tile_pool(name="res", bufs=1))
        stats = top.enter_context(tc.tile_pool(name="stats", bufs=1))

        # ---- resident tiles -------------------------------------------------
        wq_sb = res.tile([128, 8, JC], F32R, tag="wq")
        wk_sb = res.tile([128, 8, JC], F32R, tag="wk")
        wv_sb = res.tile([128, 8, JC], F32R, tag="wv")
        nc.sync.dma_start(wq_sb, wqt_d[:, :].rearrange("(t p) j -> p t j", p=128))
        nc.sync.dma_start(wk_sb, wkt_d[:, :].rearrange("(t p) j -> p t j", p=128))
        nc.sync.dma_start(wv_sb, wvt_d[:, :].rearrange("(t p) j -> p t j", p=128))
        wo_sb = []
        for p2 in range(2):
            t = res.tile([128, D], F32R, tag=f"wo{p2}", name=f"wo{p2}")
            nc.sync.dma_start(t, wot_d[128 * p2 : 128 * (p2 + 1), :])
            wo_sb.append(t)
        maska = res.tile([128, 256], BF16, tag="maska")
        maskb = res.tile([128, 256], BF16, tag="maskb")
        ident = res.tile([128, 128], BF16, tag="ident")
        nc.sync.dma_start(maska, maska_d[:, :])
        nc.sync.dma_start(maskb, maskb_d[:, :])
        nc.sync.dma_start(ident, ident_d[:, :])

        # projected tensors (resident through attention), segmented 512-wide
        qseg = [[res.tile([128, 512], F32R, tag=f"qts{i}{c}", name=f"qts{i}{c}")
                 for c in range(4)] for i in range(2)]
        kseg = [[res.tile([128, 512], F32R, tag=f"kts{i}{c}", name=f"kts{i}{c}")
                 for c in range(4)] for i in range(2)]
        vu = [res.tile([128, JC], BF16, tag=f"vu{i}", name=f"vu{i}") for i in range(NQT)]
        ctxseg = [[res.tile([128, 512], F32R, tag=f"ctx{i}{c}", name=f"ctx{i}{c}")
                   for c in range(4)] for i in range(2)]

        def _one_pass(_rep):
            # single fused stage: projections interleaved with attention
            with ExitStack() as stage_c:
                xpool = stage_c.enter_context(tc.tile_pool(name=f"xt{_rep}", bufs=1))
                ppool = stage_c.enter_context(tc.tile_pool(name=f"pp{_rep}", bufs=1))
                ptp = stage_c.enter_context(tc.tile_pool(name=f"ptp{_rep}", bufs=1))
                obp = stage_c.enter_context(tc.tile_pool(name=f"obp{_rep}", bufs=1))
                pss_p = stage_c.enter_context(
                    tc.tile_pool(name=f"pss{_rep}", bufs=1, space="PSUM"))
                pst_p = stage_c.enter_context(
                    tc.tile_pool(name=f"pst{_rep}", bufs=1, space="PSUM"))
                psc_p = stage_c.enter_context(
                    tc.tile_pool(name=f"psc{_rep}", bufs=1, space="PSUM"))
                pso_p = stage_c.enter_context(
                    tc.tile_pool(name=f"pso{_rep}", bufs=1, space="PSUM"))

                ncopy = 0  # ptsb copy engine round-robin

                def proj_chunk(ch):
                    sl = slice(ch * 512, (ch + 1) * 512)
                    for kind, xd in (("k", xtk_d), ("q", xtq_d), ("v", xtv_d)):
                        xc = xpool.tile([128, 8, 512], F32R, tag="xc", bufs=2,
                                        name="xc")
                        nc.sync.dma_start(
                            xc, xd[:, sl].rearrange("(t p) s -> p t s", p=128)
                        )
                        ps = pss_p.tile([128, 1024], F32, tag="pss",
                                        bufs=_PSS_BUFS, name=f"psj{kind}")
                        if kind in ("q", "k"):
                            wsb = wq_sb if kind == "q" else wk_sb
                            dst = qseg if kind == "q" else kseg
                            for jt in range(2):
                                for dt in range(8):
                                    nc.tensor.matmul(
                                        ps[:, 512 * jt : 512 * (jt + 1)],
                                        wsb[:, dt, 128 * jt : 128 * (jt + 1)],
                                        xc[:, dt, :],
                                        start=(dt == 0),
                                        stop=(dt == 7),
                                    )
                            for jt in range(2):
                                psl = ps[:, 512 * jt : 512 * (jt + 1)]
                                if (ch + jt) % 2 == 0:
                                    nc.vector.tensor_copy(dst[jt][ch], psl)
                                else:
                                    nc.scalar.copy(dst[jt][ch], psl)
                        else:
                            for st in range(4):
                                for dt in range(8):
                                    nc.tensor.matmul(
                                        ps[:, 256 * st : 256 * (st + 1)],
                                        xc[:, dt, st * 128 : (st + 1) * 128],
                                        wv_sb[:, dt, :],
                                        start=(dt == 0),
                                        stop=(dt == 7),
                                    )
                            for st in range(4):
                                psl = ps[:, 256 * st : 256 * (st + 1)]
                                if st % 2 == 0:
                                    nc.vector.tensor_copy(vu[4 * ch + st], psl)
                                else:
                                    nc.scalar.copy(vu[4 * ch + st], psl)

                def emit_scores(u, p):
                    ptiles = {}
                    for sq in range(4):
                            qi = 4 * u + sq
                            W = 128 * (qi + 1)
                            even = (qi % 2 == 0)
                            Wp = W + 128 if even else W
                            msk = maskb if even else maska
                            nreg = 2 if Wp > 1024 else 1
                            winoff = Wp - 256
                            for h in range(2):
                                hsl = slice(64 * h, 64 * (h + 1))
                                qstat = qseg[p][u][hsl, 128 * sq : 128 * sq + 128]
                                pt = ppool.tile([128, 2048], BF16, tag="P",
                                                bufs=_PT_BUFS, name=f"P{p}{sq}{h}")
                                ptiles[(h, sq)] = (pt, Wp)
                                stt = stats.tile([128, 8], F32, tag="st",
                                                 bufs=24, name="stt")
                                regions = []
                                for r in range(nreg):
                                    lo = 1024 * r
                                    hi = min(1024 * (r + 1), Wp)
                                    wr = hi - lo
                                    ps = pss_p.tile([128, 1024], F32, tag="pss",
                                                    bufs=_PSS_BUFS, name="pss")
                                    regions.append((ps, lo, wr))
                                    a = lo
                                    end_plain = min(hi, winoff)
                                    while a < end_plain:
                                        w = 512 if end_plain - a >= 512 else end_plain - a
                                        nc.tensor.matmul(
                                            ps[:, a - lo : a - lo + w],
                                            qstat,
                                            kseg[p][a // 512][hsl, a % 512 : a % 512 + w],
                                            start=True, stop=True,
                                            tile_position=(64 * h, 0),
                                        )
                                        a += w
                                    if hi > winoff:
                                        wo = winoff - lo
                                        nc.tensor.matmul(
                                            ps[:, wo : wo + 256], ident, msk,
                                            start=True, stop=False,
                                            skip_group_check=True,
                                        )
                                        ko = winoff % 512
                                        nc.tensor.matmul(
                                            ps[:, wo : wo + 256],
                                            qstat,
                                            kseg[p][winoff // 512][hsl, ko : ko + 256],
                                            start=False, stop=True,
                                            tile_position=(64 * h, 0),
                                            skip_group_check=True,
                                        )
                                    nc.vector.reduce_max(
                                        out=stt[:, r : r + 1], in_=ps[:, 0:wr],
                                        axis=AX, negate=True,
                                    )
                                if nreg == 2:
                                    nc.vector.tensor_tensor(
                                        stt[:, 2:3], stt[:, 0:1], stt[:, 1:2],
                                        op=MIN,
                                    )
                                    bc = 2
                                else:
                                    bc = 0
                                for r, (ps, lo, wr) in enumerate(regions):
                                    nc.scalar.activation(
                                        out=pt[:, lo : lo + wr],
                                        in_=ps[:, 0:wr],
                                        func=EXP,
                                        bias=stt[:, bc : bc + 1],
                                        scale=1.0,
                                        accum_out=stt[:, 4 + r : 5 + r],
                                    )
                                if nreg == 2:
                                    nc.vector.tensor_add(
                                        stt[:, 6:7], stt[:, 4:5], stt[:, 5:6]
                                    )
                                    zc = 6
                                else:
                                    zc = 4
                                nc.vector.reciprocal(stt[:, 7:8], stt[:, zc : zc + 1])
                                nc.vector.tensor_scalar_mul(
                                    pt[:, 0:Wp], pt[:, 0:Wp], stt[:, 7:8]
                                )
                    return ptiles

                def emit_tail(u, p, ptiles):
                    # transposes + PV for this (pair, supertile)
                    nonlocal ncopy
                    psc = psc_p.tile([128, 512], F32, tag="psc", bufs=1,
                                     name=f"psc{p}{u}")
                    for t in range(4 * u + 4):
                        vstart = max(0, t - 4 * u)
                        csl = slice(128 * vstart, 512)
                        for h in range(2):
                            pstile = pst_p.tile([128, 512], BF16, tag="pst",
                                                bufs=2, name="pst")
                            for sq in range(vstart, 4):
                                nc.tensor.transpose(
                                    pstile[:, 128 * sq : 128 * (sq + 1)],
                                    ptiles[(h, sq)][0][:, 128 * t : 128 * (t + 1)],
                                    ident,
                                )
                            ptsb = ptp.tile([128, 512], BF16, tag="pt", bufs=3,
                                            name="ptsb")
                            if _PTSB_PAT[ncopy % len(_PTSB_PAT)] == 0:
                                nc.vector.tensor_copy(ptsb[:, csl], pstile[:, csl])
                            else:
                                nc.scalar.copy(ptsb[:, csl], pstile[:, csl])
                            ncopy += 1
                            nc.tensor.matmul(
                                psc[64 * h : 64 * (h + 1), csl],
                                vu[t][:, 64 * (2 * p + h) : 64 * (2 * p + h + 1)],
                                ptsb[:, csl],
                                start=(t == 0),
                                stop=(t == 4 * u + 3),
                                tile_position=(0, 64 * h),
                                skip_group_check=True,
                            )
                    if u % 2 == 0:
                        nc.vector.tensor_copy(ctxseg[p][u], psc)
                    else:
                        nc.scalar.copy(ctxseg[p][u], psc)
                    if p != 1:
                        return
                    # output projection for the four finished s-tiles
                    for st_ in range(4 * u, 4 * u + 4):
                        ssl = slice(128 * st_, 128 * (st_ + 1))
                        csl_ = slice(128 * (st_ % 4), 128 * (st_ % 4) + 128)
                        for oc in range(2):
                            osl = slice(512 * oc, 512 * (oc + 1))
                            pso = pso_p.tile([128, 512], F32, tag="pso", bufs=1,
                                             name="pso")
                            nc.tensor.matmul(pso, ctxseg[0][u][:, csl_],
                                             wo_sb[0][:, osl],
                                             start=True, stop=False)
                            nc.tensor.matmul(pso, ctxseg[1][u][:, csl_],
                                             wo_sb[1][:, osl],
                                             start=False, stop=True)
                            osb = obp.tile([128, 512], F32, tag="ob", bufs=3,
                                           name="osb")
                            if oc == 0:
                                nc.vector.tensor_copy(osb, pso)
                            else:
                                nc.scalar.copy(osb, pso)
                            nc.gpsimd.dma_start(out_d[ssl, osl], osb)

                # software pipeline: proj chunk u feeds attention round u;
                # scores of group g+1 overlap tail of group g
                pending = None
                for u in range(NU4):
                    proj_chunk(u)
                    for p in range(2):
                        ptiles = emit_scores(u, p)
                        if pending is not None:
                            emit_tail(*pending)
                        pending = (u, p, ptiles)
                emit_tail(*pending)

        for _rep in range(reps):
            if _rep:
                tc.strict_bb_all_engine_barrier()
            _one_pass(_rep)

    nc.compile()
    return nc


def _get_nc(reps=1):
    key = ("nc", reps)
    if key not in _cached:
        _cached[key] = _build_nc(reps)
    return _cached[key]


def _fp22(a):
    """Truncate fp32 to fp22 (e8m13) as the PE's float32r datapath does."""
    a = np.ascontiguousarray(a, dtype=np.float32)
    a.view(np.uint32)[...] &= np.uint32(0xFFFFFC00)
    return a


def _host_inputs(query, key, value, Wq, Wk, Wv, Wo):
    """Build the 8 per-core input dicts (host-side transposes/slices)."""
    f32 = np.float32
    xt = {}
    for b in range(B):
        xt[("q", b)] = _fp22(query[b].T)
        xt[("k", b)] = _fp22(key[b].T)
        xt[("v", b)] = _fp22(value[b].T)
    import ml_dtypes

    q_ar = np.arange(128)[:, None]
    j_ar = np.arange(128)[None, :]
    tri = np.where(j_ar <= q_ar, 0.0, _MASKVAL).astype(f32)
    maska = np.concatenate([np.zeros((128, 128), f32), tri], axis=1)
    maskb = np.concatenate([tri, np.full((128, 128), _MASKVAL, f32)], axis=1)
    maska = maska.astype(ml_dtypes.bfloat16)
    maskb = maskb.astype(ml_dtypes.bfloat16)
    ident = np.eye(128).astype(ml_dtypes.bfloat16)
    in_maps = []
    for c in range(8):
        b, g = c // 4, c % 4
        jsl = slice(JC * g, JC * (g + 1))
        in_maps.append(
            {
                "xtq": xt[("q", b)],
                "xtk": xt[("k", b)],
                "xtv": xt[("v", b)],
                "wqt": _fp22(_SCALE * Wq[jsl, :].T),
                "wkt": _fp22(Wk[jsl, :].T),
                "wvt": _fp22(Wv[jsl, :].T),
                "wot": _fp22(Wo[:, jsl].T),
                "maska": maska,
                "maskb": maskb,
                "ident": ident,
            }
        )
    return in_maps


def _numpy_fallback(query, key, value, mask, Wq, Wk, Wv, Wo):
    """Exact (chunked) numpy path for non-causal masks."""
    out = np.empty((B, S, D), dtype=np.float32)
    q = (query @ Wq.T).reshape(B, S, H, DK).transpose(0, 2, 1, 3)
    k = (key @ Wk.T).reshape(B, S, H, DK).transpose(0, 2, 1, 3)
    v = (value @ Wv.T).reshape(B, S, H, DK).transpose(0, 2, 1, 3)
    for b in range(B):
        ctx = np.empty((H, S, DK), dtype=np.float32)
        mb = mask[b] == 0
        for h in range(H):
            s = (q[b, h] @ k[b, h].T) * _SCALE
            s[mb] = np.finfo(np.float32).min
            s -= s.max(axis=1, keepdims=True)
            np.exp(s, out=s)
            s /= s.sum(axis=1, keepdims=True)
            ctx[h] = s @ v[b, h]
        out[b] = ctx.transpose(1, 0, 2).reshape(S, D) @ Wo.T
    return out


def kernel(query, key, value, mask, Wq, Wk, Wv, Wo):
    query = np.asarray(query, dtype=np.float32)
    key = np.asarray(key, dtype=np.float32)
    value = np.asarray(value, dtype=np.float32)
    mask = np.asarray(mask)
    Wq, Wk, Wv, Wo = (np.asarray(w, dtype=np.float32) for w in (Wq, Wk, Wv, Wo))

    tril = np.tril(np.ones((S, S), dtype=mask.dtype))
    if not all(np.array_equal(mask[b], tril) for b in range(B)):
        return _numpy_fallback(query, key, value, mask, Wq, Wk, Wv, Wo)

    from concourse.bass_utils import run_bass_kernel_spmd

    nc = _get_nc()
    in_maps = _host_inputs(query, key, value, Wq, Wk, Wv, Wo)
    res = run_bass_kernel_spmd(nc, in_maps, core_ids=list(range(8)))
    outs = [r["out"] for r in res.results]
    full = np.empty((B, S, D), dtype=np.float32)
    for b in range(B):
        full[b] = outs[4 * b] + outs[4 * b + 1] + outs[4 * b + 2] + outs[4 * b + 3]
    return full


# revision 16
# speedup vs baseline: 3.0055x; 1.9254x over previous
"""Multi-head causal attention (B=2, S=2048, D=1024, H=16) on 8 trn2 NeuronCores.

Sharding: data-parallel over batch (2) x tensor-parallel over heads (4 groups of
4 heads).  Core c = 4*b + g handles batch b, heads [4g, 4g+4).  Each core
computes a partial output  ctx_g @ Wo_g.T  [2048, 1024]; the host sums the 4
partials per batch.

Within-core dataflow (v2):
  qT,kT = W @ X.T        float32r matmuls; Wq pre-scaled by sqrt(dk)=8 on host
                         so scores come out of the PE already scaled
  v     = X @ Wv.T       natural [s, dv], stored bf16
  S     = qT.T @ kT      per 128-row query tile, contiguous 2-bank PSUM regions
                         (<=1024 wide); causal mask + padding written by a PE
                         matmul (ident x mask-tile) accumulated under the
                         diagonal score matmul; all score matmuls >=256 moving
  m     = rowmax         ONE negated reduce_max per region (DVE) -> exp bias
  P     = exp(S - m)     ONE exp per region on ACT, accum_out -> Z
  P    *= 1/Z            one DVE 4x bf16 pass per tile
  P_T   = PE transpose   bf16 128x128 blocks into 512-wide q supertiles
  ctxT  = v.T @ P_T      bf16, two heads col-packed, accumulated over k tiles
  out  += ctxT.T @ WoT   float32r, per s-tile, stored straight from PSUM
"""

import numpy as np

B, S, D, H = 2, 2048, 1024, 16
DK = D // H          # 64
JC = 256             # per-core projection width (4 heads * 64)
NQT = S // 128       # 16 query tiles
NU4 = S // 512       # 4 query supertiles
_SCALE = float(DK) ** 0.5  # 8.0  (folded into Wq on the host)
_MASKVAL = -1.0e30

_cached = {}

# knobs
_PTSB_PAT = (0, 1, 0, 0, 1)   # ptsb copy engine pattern: 0=DVE, 1=ACT
_PSS_BUFS = 2
_PT_BUFS = 18


def _build_nc(reps=1):
    from contextlib import ExitStack

    import concourse.mybir as mybir
    import concourse.tile as tile
    from concourse import bacc

    F32 = mybir.dt.float32
    F32R = mybir.dt.float32r
    BF16 = mybir.dt.bfloat16
    EXP = mybir.ActivationFunctionType.Exp
    AX = mybir.AxisListType.X
    MIN = mybir.AluOpType.min

    nc = bacc.Bacc("TRN2", target_bir_lowering=False)

    xtq_d = nc.dram_tensor("xtq", [D, S], F32R, kind="ExternalInput")
    xtk_d = nc.dram_tensor("xtk", [D, S], F32R, kind="ExternalInput")
    xtv_d = nc.dram_tensor("xtv", [D, S], F32R, kind="ExternalInput")
    wqt_d = nc.dram_tensor("wqt", [D, JC], F32R, kind="ExternalInput")
    wkt_d = nc.dram_tensor("wkt", [D, JC], F32R, kind="ExternalInput")
    wvt_d = nc.dram_tensor("wvt", [D, JC], F32R, kind="ExternalInput")
    wot_d = nc.dram_tensor("wot", [JC, D], F32R, kind="ExternalInput")
    maska_d = nc.dram_tensor("maska", [128, 256], BF16, kind="ExternalInput")
    maskb_d = nc.dram_tensor("maskb", [128, 256], BF16, kind="ExternalInput")
    ident_d = nc.dram_tensor("ident", [128, 128], BF16, kind="ExternalInput")
    out_d = nc.dram_tensor("out", [S, D], F32, kind="ExternalOutput")

    with tile.TileContext(nc) as tc, ExitStack() as top:
        res = top.enter_context(tc.tile_pool(name="res", bufs=1))
        stats = top.enter_context(tc.tile_pool(name="stats", bufs=1))

        # ---- resident tiles -------------------------------------------------
        wq_sb = res.tile([128, 8, JC], F32R, tag="wq")
        wk_sb = res.tile([128, 8, JC], F32R, tag="wk")
        wv_sb = res.tile([128, 8, JC], F32R, tag="wv")
        nc.sync.dma_start(wq_sb, wqt_d[:, :].rearrange("(t p) j -> p t j", p=128))
        nc.sync.dma_start(wk_sb, wkt_d[:, :].rearrange("(t p) j -> p t j", p=128))
        nc.sync.dma_start(wv_sb, wvt_d[:, :].rearrange("(t p) j -> p t j", p=128))
        wo_sb = []
        for p2 in range(2):
            t = res.tile([128, D], F32R, tag=f"wo{p2}", name=f"wo{p2}")
            nc.sync.dma_start(t, wot_d[128 * p2 : 128 * (p2 + 1), :])
            wo_sb.append(t)
        maska = res.tile([128, 256], BF16, tag="maska")
        maskb = res.tile([128, 256], BF16, tag="maskb")
        ident = res.tile([128, 128], BF16, tag="ident")
        nc.sync.dma_start(maska, maska_d[:, :])
        nc.sync.dma_start(maskb, maskb_d[:, :])
        nc.sync.dma_start(ident, ident_d[:, :])

        # projected tensors (resident through attention), segmented 512-wide
        qseg = [[res.tile([128, 512], F32R, tag=f"qts{i}{c}", name=f"qts{i}{c}")
                 for c in range(4)] for i in range(2)]
        kseg = [[res.tile([128, 512], F32R, tag=f"kts{i}{c}", name=f"kts{i}{c}")
                 for c in range(4)] for i in range(2)]
        vu = [res.tile([128, JC], BF16, tag=f"vu{i}", name=f"vu{i}") for i in range(NQT)]
        ctxseg = [[res.tile([128, 512], F32R, tag=f"ctx{i}{c}", name=f"ctx{i}{c}")
                   for c in range(4)] for i in range(2)]

        def _one_pass(_rep):
            # single fused stage: projections interleaved with attention
            with ExitStack() as stage_c:
                xpool = stage_c.enter_context(tc.tile_pool(name=f"xt{_rep}", bufs=1))
                ppool = stage_c.enter_context(tc.tile_pool(name=f"pp{_rep}", bufs=1))
                ptp = stage_c.enter_context(tc.tile_pool(name=f"ptp{_rep}", bufs=1))
                obp = stage_c.enter_context(tc.tile_pool(name=f"obp{_rep}", bufs=1))
                pss_p = stage_c.enter_context(
                    tc.tile_pool(name=f"pss{_rep}", bufs=1, space="PSUM"))
                pst_p = stage_c.enter_context(
                    tc.tile_pool(name=f"pst{_rep}", bufs=1, space="PSUM"))
                psc_p = stage_c.enter_context(
                    tc.tile_pool(name=f"psc{_rep}", bufs=1, space="PSUM"))
                pso_p = stage_c.enter_context(
                    tc.tile_pool(name=f"pso{_rep}", bufs=1, space="PSUM"))

                ncopy = 0  # ptsb copy engine round-robin

                def proj_chunk(ch):
                    sl = slice(ch * 512, (ch + 1) * 512)
                    for kind, xd in (("k", xtk_d), ("q", xtq_d), ("v", xtv_d)):
                        xc = xpool.tile([128, 8, 512], F32R, tag="xc", bufs=2,
                                        name="xc")
                        nc.sync.dma_start(
                            xc, xd[:, sl].rearrange("(t p) s -> p t s", p=128)
                        )
                        ps = pss_p.tile([128, 1024], F32, tag="pss",
                                        bufs=_PSS_BUFS, name=f"psj{kind}")
                        if kind in ("q", "k"):
                            wsb = wq_sb if kind == "q" else wk_sb
                            dst = qseg if kind == "q" else kseg
                            for jt in range(2):
                                for dt in range(8):
                                    nc.tensor.matmul(
                                        ps[:, 512 * jt : 512 * (jt + 1)],
                                        wsb[:, dt, 128 * jt : 128 * (jt + 1)],
                                        xc[:, dt, :],
                                        start=(dt == 0),
                                        stop=(dt == 7),
                                    )
                            for jt in range(2):
                                psl = ps[:, 512 * jt : 512 * (jt + 1)]
                                if (ch + jt) % 2 == 0:
                                    nc.vector.tensor_copy(dst[jt][ch], psl)
                                else:
                                    nc.scalar.copy(dst[jt][ch], psl)
                        else:
                            for st in range(4):
                                for dt in range(8):
                                    nc.tensor.matmul(
                                        ps[:, 256 * st : 256 * (st + 1)],
                                        xc[:, dt, st * 128 : (st + 1) * 128],
                                        wv_sb[:, dt, :],
                                        start=(dt == 0),
                                        stop=(dt == 7),
                                    )
                            for st in range(4):
                                psl = ps[:, 256 * st : 256 * (st + 1)]
                                if st % 2 == 0:
                                    nc.vector.tensor_copy(vu[4 * ch + st], psl)
                                else:
                                    nc.scalar.copy(vu[4 * ch + st], psl)

                def emit_scores(u, p):
                    ptiles = {}
                    for sq in range(4):
                            qi = 4 * u + sq
                            W = 128 * (qi + 1)
                            even = (qi % 2 == 0)
                            Wp = W + 128 if even else W
                            msk = maskb if even else maska
                            nreg = 2 if Wp > 1024 else 1
                            winoff = Wp - 256
                            for h in range(2):
                                hsl = slice(64 * h, 64 * (h + 1))
                                qstat = qseg[p][u][hsl, 128 * sq : 128 * sq + 128]
                                pt = ppool.tile([128, 2048], BF16, tag="P",
                                                bufs=_PT_BUFS, name=f"P{p}{sq}{h}")
                                ptiles[(h, sq)] = (pt, Wp)
                                stt = stats.tile([128, 8], F32, tag="st",
                                                 bufs=24, name="stt")
                                regions = []
                                for r in range(nreg):
                                    lo = 1024 * r
                                    hi = min(1024 * (r + 1), Wp)
                                    wr = hi - lo
                                    ps = pss_p.tile([128, 1024], F32, tag="pss",
                                                    bufs=_PSS_BUFS, name="pss")
                                    regions.append((ps, lo, wr))
                                    a = lo
                                    end_plain = min(hi, winoff)
                                    while a < end_plain:
                                        w = 512 if end_plain - a >= 512 else end_plain - a
                                        nc.tensor.matmul(
                                            ps[:, a - lo : a - lo + w],
                                            qstat,
                                            kseg[p][a // 512][hsl, a % 512 : a % 512 + w],
                                            start=True, stop=True,
                                            tile_position=(64 * h, 0),
                                        )
                                        a += w
                                    if hi > winoff:
                                        wo = winoff - lo
                                        nc.tensor.matmul(
                                            ps[:, wo : wo + 256], ident, msk,
                                            start=True, stop=False,
                                            skip_group_check=True,
                                        )
                                        ko = winoff % 512
                                        nc.tensor.matmul(
                                            ps[:, wo : wo + 256],
                                            qstat,
                                            kseg[p][winoff // 512][hsl, ko : ko + 256],
                                            start=False, stop=True,
                                            tile_position=(64 * h, 0),
                                            skip_group_check=True,
                                        )
                                    nc.vector.reduce_max(
                                        out=stt[:, r : r + 1], in_=ps[:, 0:wr],
                                        axis=AX, negate=True,
                                    )
                                if nreg == 2:
                                    nc.vector.tensor_tensor(
                                        stt[:, 2:3], stt[:, 0:1], stt[:, 1:2],
                                        op=MIN,
                                    )
                                    bc = 2
                                else:
                                    bc = 0
                                for r, (ps, lo, wr) in enumerate(regions):
                                    nc.scalar.activation(
                                        out=pt[:, lo : lo + wr],
                                        in_=ps[:, 0:wr],
                                        func=EXP,
                                        bias=stt[:, bc : bc + 1],
                                        scale=1.0,
                                        accum_out=stt[:, 4 + r : 5 + r],
                                    )
                                if nreg == 2:
                                    nc.gpsimd.tensor_add(
                                        stt[:, 6:7], stt[:, 4:5], stt[:, 5:6]
                                    )
                                    zc = 6
                                else:
                                    zc = 4
                                nc.vector.reciprocal(stt[:, 7:8], stt[:, zc : zc + 1])
                                nc.vector.tensor_scalar_mul(
                                    pt[:, 0:Wp], pt[:, 0:Wp], stt[:, 7:8]
                                )
                    return ptiles

                def emit_tail(u, p, ptiles):
                    # transposes + PV for this (pair, supertile)
                    nonlocal ncopy
                    psc = psc_p.tile([128, 512], F32, tag="psc", bufs=1,
                                     name=f"psc{p}{u}")
                    for t in range(4 * u + 4):
                        vstart = max(0, t - 4 * u)
                        pstile = pst_p.tile([128, 1024], BF16, tag="pst",
                                            bufs=1, name="pst")
                        for h in range(2):
                            for sq in range(vstart, 4):
                                nc.tensor.transpose(
                                    pstile[:, 512 * h + 128 * sq : 512 * h + 128 * (sq + 1)],
                                    ptiles[(h, sq)][0][:, 128 * t : 128 * (t + 1)],
                                    ident,
                                )
                        ptsb = ptp.tile([128, 1024], BF16, tag="pt", bufs=3,
                                        name="ptsb")
                        if vstart == 0:
                            slices = [slice(0, 1024)]
                        else:
                            slices = [
                                slice(512 * h + 128 * vstart, 512 * (h + 1))
                                for h in range(2)
                            ]
                        for wsl in slices:
                            if _PTSB_PAT[ncopy % len(_PTSB_PAT)] == 0:
                                nc.vector.tensor_copy(ptsb[:, wsl], pstile[:, wsl])
                            else:
                                nc.scalar.copy(ptsb[:, wsl], pstile[:, wsl])
                            ncopy += 1
                        for h in range(2):
                            csl = slice(128 * vstart, 512)
                            nc.tensor.matmul(
                                psc[64 * h : 64 * (h + 1), csl],
                                vu[t][:, 64 * (2 * p + h) : 64 * (2 * p + h + 1)],
                                ptsb[:, 512 * h :][:, csl],
                                start=(t == 0),
                                stop=(t == 4 * u + 3),
                                tile_position=(0, 64 * h),
                                skip_group_check=True,
                            )
                    if u % 2 == 0:
                        nc.vector.tensor_copy(ctxseg[p][u], psc)
                    else:
                        nc.scalar.copy(ctxseg[p][u], psc)
                    if p != 1:
                        return
                    # output projection for the four finished s-tiles
                    for st_ in range(4 * u, 4 * u + 4):
                        ssl = slice(128 * st_, 128 * (st_ + 1))
                        csl_ = slice(128 * (st_ % 4), 128 * (st_ % 4) + 128)
                        for oc in range(2):
                            osl = slice(512 * oc, 512 * (oc + 1))
                            pso = pso_p.tile([128, 512], F32, tag="pso", bufs=1,
                                             name="pso")
                            nc.tensor.matmul(pso, ctxseg[0][u][:, csl_],
                                             wo_sb[0][:, osl],
                                             start=True, stop=False)
                            nc.tensor.matmul(pso, ctxseg[1][u][:, csl_],
                                             wo_sb[1][:, osl],
                                             start=False, stop=True)
                            osb = obp.tile([128, 512], F32, tag="ob", bufs=3,
                                           name="osb")
                            if oc == 0:
                                nc.vector.tensor_copy(osb, pso)
                            else:
                                nc.scalar.copy(osb, pso)
                            nc.gpsimd.dma_start(out_d[ssl, osl], osb)

                # software pipeline: proj chunk u feeds attention round u;
                # scores of group g+1 overlap tail of group g
                pending = None
                for u in range(NU4):
                    proj_chunk(u)
                    for p in range(2):
                        ptiles = emit_scores(u, p)
                        if pending is not None:
                            emit_tail(*pending)
                        pending = (u, p, ptiles)
                emit_tail(*pending)

        for _rep in range(reps):
            if _rep:
                tc.strict_bb_all_engine_barrier()
            _one_pass(_rep)

    nc.compile()
    return nc


def _get_nc(reps=1):
    key = ("nc", reps)
    if key not in _cached:
        _cached[key] = _build_nc(reps)
    return _cached[key]


def _fp22(a):
    """Truncate fp32 to fp22 (e8m13) as the PE's float32r datapath does."""
    a = np.ascontiguousarray(a, dtype=np.float32)
    a.view(np.uint32)[...] &= np.uint32(0xFFFFFC00)
    return a


def _host_inputs(query, key, value, Wq, Wk, Wv, Wo):
    """Build the 8 per-core input dicts (host-side transposes/slices)."""
    f32 = np.float32
    xt = {}
    for b in range(B):
        xt[("q", b)] = _fp22(query[b].T)
        xt[("k", b)] = _fp22(key[b].T)
        xt[("v", b)] = _fp22(value[b].T)
    import ml_dtypes

    q_ar = np.arange(128)[:, None]
    j_ar = np.arange(128)[None, :]
    tri = np.where(j_ar <= q_ar, 0.0, _MASKVAL).astype(f32)
    maska = np.concatenate([np.zeros((128, 128), f32), tri], axis=1)
    maskb = np.concatenate([tri, np.full((128, 128), _MASKVAL, f32)], axis=1)
    maska = maska.astype(ml_dtypes.bfloat16)
    maskb = maskb.astype(ml_dtypes.bfloat16)
    ident = np.eye(128).astype(ml_dtypes.bfloat16)
    in_maps = []
    for c in range(8):
        b, g = c // 4, c % 4
        jsl = slice(JC * g, JC * (g + 1))
        in_maps.append(
            {
                "xtq": xt[("q", b)],
                "xtk": xt[("k", b)],
                "xtv": xt[("v", b)],
                "wqt": _fp22(_SCALE * Wq[jsl, :].T),
                "wkt": _fp22(Wk[jsl, :].T),
                "wvt": _fp22(Wv[jsl, :].T),
                "wot": _fp22(Wo[:, jsl].T),
                "maska": maska,
                "maskb": maskb,
                "ident": ident,
            }
        )
    return in_maps


def _numpy_fallback(query, key, value, mask, Wq, Wk, Wv, Wo):
    """Exact (chunked) numpy path for non-causal masks."""
    out = np.empty((B, S, D), dtype=np.float32)
    q = (query @ Wq.T).reshape(B, S, H, DK).transpose(0, 2, 1, 3)
    k = (key @ Wk.T).reshape(B, S, H, DK).transpose(0, 2, 1, 3)
    v = (value @ Wv.T).reshape(B, S, H, DK).transpose(0, 2, 1, 3)
    for b in range(B):
        ctx = np.empty((H, S, DK), dtype=np.float32)
        mb = mask[b] == 0
        for h in range(H):
            s = (q[b, h] @ k[b, h].T) * _SCALE
            s[mb] = np.finfo(np.float32).min
            s -= s.max(axis=1, keepdims=True)
            np.exp(s, out=s)
            s /= s.sum(axis=1, keepdims=True)
            ctx[h] = s @ v[b, h]
        out[b] = ctx.transpose(1, 0, 2).reshape(S, D) @ Wo.T
    return out


def kernel(query, key, value, mask, Wq, Wk, Wv, Wo):
    query = np.asarray(query, dtype=np.float32)
    key = np.asarray(key, dtype=np.float32)
    value = np.asarray(value, dtype=np.float32)
    mask = np.asarray(mask)
    Wq, Wk, Wv, Wo = (np.asarray(w, dtype=np.float32) for w in (Wq, Wk, Wv, Wo))

    tril = np.tril(np.ones((S, S), dtype=mask.dtype))
    if not all(np.array_equal(mask[b], tril) for b in range(B)):
        return _numpy_fallback(query, key, value, mask, Wq, Wk, Wv, Wo)

    from concourse.bass_utils import run_bass_kernel_spmd

    nc = _get_nc()
    in_maps = _host_inputs(query, key, value, Wq, Wk, Wv, Wo)
    res = run_bass_kernel_spmd(nc, in_maps, core_ids=list(range(8)))
    outs = [r["out"] for r in res.results]
    full = np.empty((B, S, D), dtype=np.float32)
    for b in range(B):
        full[b] = outs[4 * b] + outs[4 * b + 1] + outs[4 * b + 2] + outs[4 * b + 3]
    return full


# revision 18
# speedup vs baseline: 6.5042x; 2.1641x over previous
"""Multi-head causal attention (B=2, S=2048, D=1024, H=16) on 8 trn2 NeuronCores.

Sharding: data-parallel over batch (2) x tensor-parallel over heads (4 groups of
4 heads).  Core c = 4*b + g handles batch b, heads [4g, 4g+4).  Each core
computes a partial output  ctx_g @ Wo_g.T  [2048, 1024]; the host sums the 4
partials per batch.

Within-core dataflow (v2):
  qT,kT = W @ X.T        float32r matmuls; Wq pre-scaled by sqrt(dk)=8 on host
                         so scores come out of the PE already scaled
  v     = X @ Wv.T       natural [s, dv], stored bf16
  S     = qT.T @ kT      per 128-row query tile, contiguous 2-bank PSUM regions
                         (<=1024 wide); causal mask + padding written by a PE
                         matmul (ident x mask-tile) accumulated under the
                         diagonal score matmul; all score matmuls >=256 moving
  m     = rowmax         ONE negated reduce_max per region (DVE) -> exp bias
  P     = exp(S - m)     ONE exp per region on ACT, accum_out -> Z
  P    *= 1/Z            one DVE 4x bf16 pass per tile
  P_T   = PE transpose   bf16 128x128 blocks into 512-wide q supertiles
  ctxT  = v.T @ P_T      bf16, two heads col-packed, accumulated over k tiles
  out  += ctxT.T @ WoT   float32r, per s-tile, stored straight from PSUM
"""

import numpy as np

B, S, D, H = 2, 2048, 1024, 16
DK = D // H          # 64
JC = 256             # per-core projection width (4 heads * 64)
NQT = S // 128       # 16 query tiles
NU4 = S // 512       # 4 query supertiles
_SCALE = float(DK) ** 0.5  # 8.0  (folded into Wq on the host)
_MASKVAL = -1.0e30

_cached = {}

# knobs
_PTSB_PAT = (0, 1, 0, 0, 1)   # ptsb copy engine pattern: 0=DVE, 1=ACT
_PSS_BUFS = 2
_PT_BUFS = 18


def _build_nc(reps=1):
    from contextlib import ExitStack

    import concourse.mybir as mybir
    import concourse.tile as tile
    from concourse import bacc

    F32 = mybir.dt.float32
    F32R = mybir.dt.float32r
    BF16 = mybir.dt.bfloat16
    EXP = mybir.ActivationFunctionType.Exp
    AX = mybir.AxisListType.X
    MIN = mybir.AluOpType.min

    nc = bacc.Bacc("TRN2", target_bir_lowering=False)

    xtq_d = nc.dram_tensor("xtq", [D, S], F32R, kind="ExternalInput")
    xtk_d = nc.dram_tensor("xtk", [D, S], F32R, kind="ExternalInput")
    xtv_d = nc.dram_tensor("xtv", [D, S], F32R, kind="ExternalInput")
    wqt_d = nc.dram_tensor("wqt", [D, JC], F32R, kind="ExternalInput")
    wkt_d = nc.dram_tensor("wkt", [D, JC], F32R, kind="ExternalInput")
    wvt_d = nc.dram_tensor("wvt", [D, JC], F32R, kind="ExternalInput")
    wot_d = nc.dram_tensor("wot", [JC, D], F32R, kind="ExternalInput")
    maska_d = nc.dram_tensor("maska", [128, 256], BF16, kind="ExternalInput")
    maskb_d = nc.dram_tensor("maskb", [128, 256], BF16, kind="ExternalInput")
    ident_d = nc.dram_tensor("ident", [128, 128], BF16, kind="ExternalInput")
    out_d = nc.dram_tensor("out", [S, D], F32, kind="ExternalOutput")

    with tile.TileContext(nc) as tc, ExitStack() as top:
        res = top.enter_context(tc.tile_pool(name="res", bufs=1))
        stats = top.enter_context(tc.tile_pool(name="stats", bufs=1))

        # ---- resident tiles -------------------------------------------------
        wq_sb = res.tile([128, 8, JC], F32R, tag="wq")
        wk_sb = res.tile([128, 8, JC], F32R, tag="wk")
        wv_sb = res.tile([128, 8, JC], F32R, tag="wv")
        nc.sync.dma_start(wq_sb, wqt_d[:, :].rearrange("(t p) j -> p t j", p=128))
        nc.sync.dma_start(wk_sb, wkt_d[:, :].rearrange("(t p) j -> p t j", p=128))
        nc.sync.dma_start(wv_sb, wvt_d[:, :].rearrange("(t p) j -> p t j", p=128))
        wo_sb = []
        for p2 in range(2):
            t = res.tile([128, D], F32R, tag=f"wo{p2}", name=f"wo{p2}")
            nc.sync.dma_start(t, wot_d[128 * p2 : 128 * (p2 + 1), :])
            wo_sb.append(t)
        maska = res.tile([128, 256], BF16, tag="maska")
        maskb = res.tile([128, 256], BF16, tag="maskb")
        ident = res.tile([128, 128], BF16, tag="ident")
        nc.sync.dma_start(maska, maska_d[:, :])
        nc.sync.dma_start(maskb, maskb_d[:, :])
        nc.sync.dma_start(ident, ident_d[:, :])

        # projected tensors (resident through attention), segmented 512-wide
        qseg = [[res.tile([128, 512], F32R, tag=f"qts{i}{c}", name=f"qts{i}{c}")
                 for c in range(4)] for i in range(2)]
        kseg = [[res.tile([128, 512], F32R, tag=f"kts{i}{c}", name=f"kts{i}{c}")
                 for c in range(4)] for i in range(2)]
        vu = [res.tile([128, JC], BF16, tag=f"vu{i}", name=f"vu{i}") for i in range(NQT)]
        ctxseg = [[res.tile([128, 512], F32R, tag=f"ctx{i}{c}", name=f"ctx{i}{c}")
                   for c in range(4)] for i in range(2)]

        def _one_pass(_rep):
            # single fused stage: projections interleaved with attention
            with ExitStack() as stage_c:
                xpool = stage_c.enter_context(tc.tile_pool(name=f"xt{_rep}", bufs=1))
                ppool = stage_c.enter_context(tc.tile_pool(name=f"pp{_rep}", bufs=1))
                ptp = stage_c.enter_context(tc.tile_pool(name=f"ptp{_rep}", bufs=1))
                obp = stage_c.enter_context(tc.tile_pool(name=f"obp{_rep}", bufs=1))
                pss_p = stage_c.enter_context(
                    tc.tile_pool(name=f"pss{_rep}", bufs=1, space="PSUM"))
                pst_p = stage_c.enter_context(
                    tc.tile_pool(name=f"pst{_rep}", bufs=1, space="PSUM"))
                psc_p = stage_c.enter_context(
                    tc.tile_pool(name=f"psc{_rep}", bufs=1, space="PSUM"))
                pso_p = stage_c.enter_context(
                    tc.tile_pool(name=f"pso{_rep}", bufs=1, space="PSUM"))

                ncopy = 0  # ptsb copy engine round-robin

                def proj_chunk(ch):
                    # 256-col subloads so the first matmuls start ~3us in
                    for kind, xd in (("k", xtk_d), ("q", xtq_d), ("v", xtv_d)):
                        ps = pss_p.tile([128, 1024], F32, tag="pss",
                                        bufs=_PSS_BUFS, name=f"psj{kind}")
                        for half in range(2):
                            sl = slice(ch * 512 + 256 * half,
                                       ch * 512 + 256 * (half + 1))
                            xc = xpool.tile([128, 8, 256], F32R, tag="xc",
                                            bufs=4, name="xc")
                            nc.sync.dma_start(
                                xc, xd[:, sl].rearrange("(t p) s -> p t s", p=128)
                            )
                            if kind in ("q", "k"):
                                wsb = wq_sb if kind == "q" else wk_sb
                                for jt in range(2):
                                    for dt in range(8):
                                        nc.tensor.matmul(
                                            ps[:, 512 * jt + 256 * half :
                                               512 * jt + 256 * (half + 1)],
                                            wsb[:, dt, 128 * jt : 128 * (jt + 1)],
                                            xc[:, dt, :],
                                            start=(dt == 0),
                                            stop=(dt == 7),
                                        )
                            else:
                                for st2 in range(2):
                                    st = 2 * half + st2
                                    for dt in range(8):
                                        nc.tensor.matmul(
                                            ps[:, 256 * st : 256 * (st + 1)],
                                            xc[:, dt, st2 * 128 : (st2 + 1) * 128],
                                            wv_sb[:, dt, :],
                                            start=(dt == 0),
                                            stop=(dt == 7),
                                        )
                        if kind in ("q", "k"):
                            dst = qseg if kind == "q" else kseg
                            # round 0: copy 256-halves separately so the first
                            # score matmuls can start before the full chunk
                            # lands
                            spans = ((0, 256), (256, 512)) if ch == 0 else ((0, 512),)
                            for jt in range(2):
                                for lo_, hi_ in spans:
                                    psl = ps[:, 512 * jt + lo_ : 512 * jt + hi_]
                                    dsl = dst[jt][ch][:, lo_:hi_]
                                    if (ch + jt) % 2 == 0:
                                        nc.vector.tensor_copy(dsl, psl)
                                    else:
                                        nc.scalar.copy(dsl, psl)
                        else:
                            for st in range(4):
                                psl = ps[:, 256 * st : 256 * (st + 1)]
                                if st % 2 == 0:
                                    nc.vector.tensor_copy(vu[4 * ch + st], psl)
                                else:
                                    nc.scalar.copy(vu[4 * ch + st], psl)

                def emit_scores(u, p):
                    ptiles = {}
                    for sq in range(4):
                            qi = 4 * u + sq
                            W = 128 * (qi + 1)
                            even = (qi % 2 == 0)
                            Wp = W + 128 if even else W
                            msk = maskb if even else maska
                            nreg = 2 if Wp > 1024 else 1
                            winoff = Wp - 256
                            for h in range(2):
                                hsl = slice(64 * h, 64 * (h + 1))
                                qstat = qseg[p][u][hsl, 128 * sq : 128 * sq + 128]
                                pt = ppool.tile([128, 2048], BF16, tag="P",
                                                bufs=_PT_BUFS, name=f"P{p}{sq}{h}")
                                ptiles[(h, sq)] = (pt, Wp)
                                stt = stats.tile([128, 8], F32, tag="st",
                                                 bufs=24, name="stt")
                                regions = []
                                for r in range(nreg):
                                    lo = 1024 * r
                                    hi = min(1024 * (r + 1), Wp)
                                    wr = hi - lo
                                    ps = pss_p.tile([128, 1024], F32, tag="pss",
                                                    bufs=_PSS_BUFS, name="pss")
                                    regions.append((ps, lo, wr))
                                    a = lo
                                    end_plain = min(hi, winoff)
                                    while a < end_plain:
                                        w = 512 if end_plain - a >= 512 else end_plain - a
                                        nc.tensor.matmul(
                                            ps[:, a - lo : a - lo + w],
                                            qstat,
                                            kseg[p][a // 512][hsl, a % 512 : a % 512 + w],
                                            start=True, stop=True,
                                            tile_position=(64 * h, 0),
                                        )
                                        a += w
                                    if hi > winoff:
                                        wo = winoff - lo
                                        nc.tensor.matmul(
                                            ps[:, wo : wo + 256], ident, msk,
                                            start=True, stop=False,
                                            skip_group_check=True,
                                        )
                                        ko = winoff % 512
                                        nc.tensor.matmul(
                                            ps[:, wo : wo + 256],
                                            qstat,
                                            kseg[p][winoff // 512][hsl, ko : ko + 256],
                                            start=False, stop=True,
                                            tile_position=(64 * h, 0),
                                            skip_group_check=True,
                                        )
                                    nc.vector.reduce_max(
                                        out=stt[:, r : r + 1], in_=ps[:, 0:wr],
                                        axis=AX, negate=True,
                                    )
                                if nreg == 2:
                                    nc.vector.tensor_tensor(
                                        stt[:, 2:3], stt[:, 0:1], stt[:, 1:2],
                                        op=MIN,
                                    )
                                    bc = 2
                                else:
                                    bc = 0
                                for r, (ps, lo, wr) in enumerate(regions):
                                    nc.scalar.activation(
                                        out=pt[:, lo : lo + wr],
                                        in_=ps[:, 0:wr],
                                        func=EXP,
                                        bias=stt[:, bc : bc + 1],
                                        scale=1.0,
                                        accum_out=stt[:, 4 + r : 5 + r],
                                    )
                                if nreg == 2:
                                    nc.gpsimd.tensor_add(
                                        stt[:, 6:7], stt[:, 4:5], stt[:, 5:6]
                                    )
                                    zc = 6
                                else:
                                    zc = 4
                                nc.vector.reciprocal(stt[:, 7:8], stt[:, zc : zc + 1])
                                nc.vector.tensor_scalar_mul(
                                    pt[:, 0:Wp], pt[:, 0:Wp], stt[:, 7:8]
                                )
                    return ptiles

                def emit_tail(u, p, ptiles):
                    # transposes + PV for this (pair, supertile)
                    nonlocal ncopy
                    psc = psc_p.tile([128, 512], F32, tag="psc", bufs=1,
                                     name=f"psc{p}{u}")
                    for t in range(4 * u + 4):
                        vstart = max(0, t - 4 * u)
                        pstile = pst_p.tile([128, 1024], BF16, tag="pst",
                                            bufs=1, name="pst")
                        for h in range(2):
                            for sq in range(vstart, 4):
                                nc.tensor.transpose(
                                    pstile[:, 512 * h + 128 * sq : 512 * h + 128 * (sq + 1)],
                                    ptiles[(h, sq)][0][:, 128 * t : 128 * (t + 1)],
                                    ident,
                                )
                        ptsb = ptp.tile([128, 1024], BF16, tag="pt", bufs=3,
                                        name="ptsb")
                        if vstart == 0:
                            slices = [slice(0, 1024)]
                        else:
                            slices = [
                                slice(512 * h + 128 * vstart, 512 * (h + 1))
                                for h in range(2)
                            ]
                        for wsl in slices:
                            if _PTSB_PAT[ncopy % len(_PTSB_PAT)] == 0:
                                nc.vector.tensor_copy(ptsb[:, wsl], pstile[:, wsl])
                            else:
                                nc.scalar.copy(ptsb[:, wsl], pstile[:, wsl])
                            ncopy += 1
                        for h in range(2):
                            csl = slice(128 * vstart, 512)
                            nc.tensor.matmul(
                                psc[64 * h : 64 * (h + 1), csl],
                                vu[t][:, 64 * (2 * p + h) : 64 * (2 * p + h + 1)],
                                ptsb[:, 512 * h :][:, csl],
                                start=(t == 0),
                                stop=(t == 4 * u + 3),
                                tile_position=(0, 64 * h),
                                skip_group_check=True,
                            )
                    if u % 2 == 0:
                        nc.vector.tensor_copy(ctxseg[p][u], psc)
                    else:
                        nc.scalar.copy(ctxseg[p][u], psc)
                    if p != 1:
                        return
                    # output projection for the four finished s-tiles
                    for st_ in range(4 * u, 4 * u + 4):
                        ssl = slice(128 * st_, 128 * (st_ + 1))
                        csl_ = slice(128 * (st_ % 4), 128 * (st_ % 4) + 128)
                        for oc in range(2):
                            osl = slice(512 * oc, 512 * (oc + 1))
                            pso = pso_p.tile([128, 512], F32, tag="pso", bufs=1,
                                             name="pso")
                            nc.tensor.matmul(pso, ctxseg[0][u][:, csl_],
                                             wo_sb[0][:, osl],
                                             start=True, stop=False)
                            nc.tensor.matmul(pso, ctxseg[1][u][:, csl_],
                                             wo_sb[1][:, osl],
                                             start=False, stop=True)
                            osb = obp.tile([128, 512], F32, tag="ob", bufs=3,
                                           name="osb")
                            if oc == 0:
                                nc.vector.tensor_copy(osb, pso)
                            else:
                                nc.scalar.copy(osb, pso)
                            nc.gpsimd.dma_start(out_d[ssl, osl], osb)

                # software pipeline: proj chunk u feeds attention round u;
                # scores of group g+1 overlap tail of group g
                pending = None
                for u in range(NU4):
                    proj_chunk(u)
                    for p in range(2):
                        ptiles = emit_scores(u, p)
                        if pending is not None:
                            emit_tail(*pending)
                        pending = (u, p, ptiles)
                emit_tail(*pending)

        for _rep in range(reps):
            if _rep:
                tc.strict_bb_all_engine_barrier()
            _one_pass(_rep)

    nc.compile()
    return nc


def _get_nc(reps=1):
    key = ("nc", reps)
    if key not in _cached:
        _cached[key] = _build_nc(reps)
    return _cached[key]


def _fp22(a):
    """Truncate fp32 to fp22 (e8m13) as the PE's float32r datapath does."""
    a = np.ascontiguousarray(a, dtype=np.float32)
    a.view(np.uint32)[...] &= np.uint32(0xFFFFFC00)
    return a


def _host_inputs(query, key, value, Wq, Wk, Wv, Wo):
    """Build the 8 per-core input dicts (host-side transposes/slices)."""
    f32 = np.float32
    xt = {}
    for b in range(B):
        xt[("q", b)] = _fp22(query[b].T)
        xt[("k", b)] = _fp22(key[b].T)
        xt[("v", b)] = _fp22(value[b].T)
    import ml_dtypes

    q_ar = np.arange(128)[:, None]
    j_ar = np.arange(128)[None, :]
    tri = np.where(j_ar <= q_ar, 0.0, _MASKVAL).astype(f32)
    maska = np.concatenate([np.zeros((128, 128), f32), tri], axis=1)
    maskb = np.concatenate([tri, np.full((128, 128), _MASKVAL, f32)], axis=1)
    maska = maska.astype(ml_dtypes.bfloat16)
    maskb = maskb.astype(ml_dtypes.bfloat16)
    ident = np.eye(128).astype(ml_dtypes.bfloat16)
    in_maps = []
    for c in range(8):
        b, g = c // 4, c % 4
        jsl = slice(JC * g, JC * (g + 1))
        in_maps.append(
            {
                "xtq": xt[("q", b)],
                "xtk": xt[("k", b)],
                "xtv": xt[("v", b)],
                "wqt": _fp22(_SCALE * Wq[jsl, :].T),
                "wkt": _fp22(Wk[jsl, :].T),
                "wvt": _fp22(Wv[jsl, :].T),
                "wot": _fp22(Wo[:, jsl].T),
                "maska": maska,
                "maskb": maskb,
                "ident": ident,
            }
        )
    return in_maps


def _numpy_fallback(query, key, value, mask, Wq, Wk, Wv, Wo):
    """Exact (chunked) numpy path for non-causal masks."""
    out = np.empty((B, S, D), dtype=np.float32)
    q = (query @ Wq.T).reshape(B, S, H, DK).transpose(0, 2, 1, 3)
    k = (key @ Wk.T).reshape(B, S, H, DK).transpose(0, 2, 1, 3)
    v = (value @ Wv.T).reshape(B, S, H, DK).transpose(0, 2, 1, 3)
    for b in range(B):
        ctx = np.empty((H, S, DK), dtype=np.float32)
        mb = mask[b] == 0
        for h in range(H):
            s = (q[b, h] @ k[b, h].T) * _SCALE
            s[mb] = np.finfo(np.float32).min
            s -= s.max(axis=1, keepdims=True)
            np.exp(s, out=s)
            s /= s.sum(axis=1, keepdims=True)
            ctx[h] = s @ v[b, h]
        out[b] = ctx.transpose(1, 0, 2).reshape(S, D) @ Wo.T
    return out


def kernel(query, key, value, mask, Wq, Wk, Wv, Wo):
    query = np.asarray(query, dtype=np.float32)
    key = np.asarray(key, dtype=np.float32)
    value = np.asarray(value, dtype=np.float32)
    mask = np.asarray(mask)
    Wq, Wk, Wv, Wo = (np.asarray(w, dtype=np.float32) for w in (Wq, Wk, Wv, Wo))

    tril = np.tril(np.ones((S, S), dtype=mask.dtype))
    if not all(np.array_equal(mask[b], tril) for b in range(B)):
        return _numpy_fallback(query, key, value, mask, Wq, Wk, Wv, Wo)

    from concourse.bass_utils import run_bass_kernel_spmd

    nc = _get_nc()
    in_maps = _host_inputs(query, key, value, Wq, Wk, Wv, Wo)
    res = run_bass_kernel_spmd(nc, in_maps, core_ids=list(range(8)))
    outs = [r["out"] for r in res.results]
    full = np.empty((B, S, D), dtype=np.float32)
    for b in range(B):
        full[b] = outs[4 * b] + outs[4 * b + 1] + outs[4 * b + 2] + outs[4 * b + 3]
    return full
